# revision 2
# baseline (speedup 1.0000x reference)
"""Multi-head attention (N=2, L=2048, 16 heads x 64) on 8 TRN2 NeuronCores.

Sharding: head-parallel attention (2 heads/core, both batches), then one
8-core AllToAll to switch to sequence-parallel for the output projection.
All matmuls bf16 with fp32 accumulation; softmax in fp32 (exp on ScalarE,
denominator via a ones-column appended to V).
"""
import sys

sys.path.insert(0, "/opt/trn_rl_repo")

import numpy as np
import ml_dtypes

import concourse.bass as bass
import concourse.bacc as bacc
import concourse.mybir as mybir
import concourse.tile as tile
from concourse.bass_utils import run_bass_kernel_spmd

BF16 = ml_dtypes.bfloat16

DM = 1024      # dmodel
DK = 64        # head dim
H = 16         # heads
NB = 2         # batch
L = 2048       # seq len
R = NB * L     # combined rows
NC = 8         # cores
HPC = H // NC  # heads per core = 2
DPC = HPC * DK  # depth per core = 128

QW = 512       # q window for scores
KT = 128       # k tile
NQT = L // KT   # 16 k/q tiles per batch
NQB = L // QW   # 4 q windows per batch
CHUNK = R // NC  # 512 rows of combined axis per core

_CACHE = {}


def _classify_blocks(mask):
    """Per (qb, kt): 0=skip, 1=full, 2=partial; dedup partial patterns.

    Returns (blocks[qb][kt], m128[qi][kt] bool, patterns [n,128,QW] bf16).
    """
    mask = np.asarray(mask, dtype=bool)
    blocks = [[0] * NQT for _ in range(NQB)]
    pat_ids = {}
    pats = []
    pat_idx = [[-1] * NQT for _ in range(NQB)]
    for qb in range(NQB):
        for kt in range(NQT):
            sub = mask[qb * QW:(qb + 1) * QW, kt * KT:(kt + 1) * KT]
            if not sub.any():
                blocks[qb][kt] = 0
            elif sub.all():
                blocks[qb][kt] = 1
            else:
                blocks[qb][kt] = 2
                pat = np.ascontiguousarray(sub.T).astype(BF16)  # [128 k, QW q]
                key = pat.tobytes()
                if key not in pat_ids:
                    pat_ids[key] = len(pats)
                    pats.append(pat)
                pat_idx[qb][kt] = pat_ids[key]
    m128 = np.zeros((NQT, NQT), dtype=bool)
    for qi in range(NQT):
        for kt in range(NQT):
            m128[qi, kt] = mask[qi * KT:(qi + 1) * KT, kt * KT:(kt + 1) * KT].any()
    if not pats:
        pats.append(np.ones((KT, QW), dtype=BF16))
    return blocks, m128, np.stack(pats), pat_idx


def _build(blocks, m128, pat_idx, n_pat):
    nc = bacc.Bacc("TRN2", target_bir_lowering=False, debug=False,
                   enable_asserts=False, num_devices=NC)
    f32, bf16 = mybir.dt.float32, mybir.dt.bfloat16

    xtb = nc.dram_tensor("xtb", [DM, R], bf16, kind="ExternalInput")
    ytb = nc.dram_tensor("ytb", [DM, R], bf16, kind="ExternalInput")
    wq = nc.dram_tensor("wq", [DM, DPC], bf16, kind="ExternalInput")
    wk = nc.dram_tensor("wk", [DM, DPC], bf16, kind="ExternalInput")
    wv = nc.dram_tensor("wv", [DM, HPC * 65], bf16, kind="ExternalInput")
    wo = nc.dram_tensor("wo", [DM, DM], bf16, kind="ExternalInput")
    bqd = nc.dram_tensor("bq", [DPC, 1], f32, kind="ExternalInput")
    bkd = nc.dram_tensor("bk", [DPC, 1], f32, kind="ExternalInput")
    bv1 = nc.dram_tensor("bv1", [1, HPC * 65], bf16, kind="ExternalInput")
    bod = nc.dram_tensor("bo", [DM, 1], f32, kind="ExternalInput")
    mpat = nc.dram_tensor("mpat", [n_pat, KT, QW], bf16, kind="ExternalInput")
    idn = nc.dram_tensor("idn", [128, 128], bf16, kind="ExternalInput")
    out_t = nc.dram_tensor("out_t", [DM, CHUNK], f32, kind="ExternalOutput")

    VW = 65 * HPC  # v_aug width per k-tile (both heads)

    with tile.TileContext(nc) as tc:
        with (
            tc.tile_pool(name="const", bufs=1) as cst,
            tc.tile_pool(name="xy", bufs=12) as xy,
            tc.tile_pool(name="big", bufs=1) as big,
            tc.tile_pool(name="exp", bufs=24) as expp,
            tc.tile_pool(name="sm", bufs=4) as sm,
            tc.tile_pool(name="osb", bufs=3) as osb,
            tc.tile_pool(name="pp", bufs=2, space="PSUM") as pp,
            tc.tile_pool(name="sp", bufs=2, space="PSUM") as sp,
            tc.tile_pool(name="avp", bufs=2, space="PSUM") as avp,
            tc.tile_pool(name="trp", bufs=2, space="PSUM") as trp,
            tc.tile_pool(name="dram", bufs=1, space="DRAM") as dram,
        ):
            # ---- constants to SBUF ----
            wq_sb = cst.tile([128, 8 * DPC], bf16)
            wk_sb = cst.tile([128, 8 * DPC], bf16)
            wv_sb = cst.tile([128, 8 * VW], bf16)
            wo_sb = cst.tile([128, 8 * DM], bf16)
            for dt in range(8):
                nc.sync.dma_start(wq_sb[:, dt * DPC:(dt + 1) * DPC], wq[dt * 128:(dt + 1) * 128, :])
                nc.sync.dma_start(wk_sb[:, dt * DPC:(dt + 1) * DPC], wk[dt * 128:(dt + 1) * 128, :])
                nc.sync.dma_start(wv_sb[:, dt * VW:(dt + 1) * VW], wv[dt * 128:(dt + 1) * 128, :])
                nc.sync.dma_start(wo_sb[:, dt * DM:(dt + 1) * DM], wo[dt * 128:(dt + 1) * 128, :])
            bq_sb = cst.tile([DPC, 1], f32)
            bk_sb = cst.tile([DPC, 1], f32)
            nc.sync.dma_start(bq_sb[:], bqd[:])
            nc.sync.dma_start(bk_sb[:], bkd[:])
            bv1_sb = cst.tile([1, VW], bf16)
            nc.sync.dma_start(bv1_sb[:], bv1[:])
            bo_sb = cst.tile([128, 8], f32)
            for mt in range(8):
                nc.sync.dma_start(bo_sb[:, mt:mt + 1], bod[mt * 128:(mt + 1) * 128, :])
            idn_sb = cst.tile([128, 128], bf16)
            nc.sync.dma_start(idn_sb[:], idn[:])
            mpat_sb = cst.tile([KT, n_pat * QW], bf16)
            for p in range(n_pat):
                nc.sync.dma_start(mpat_sb[:, p * QW:(p + 1) * QW], mpat[p])
            ones_row = cst.tile([1, 128], bf16)
            nc.vector.memset(ones_row[:], 1.0)

            qT = big.tile([DPC, R], bf16)
            kT = big.tile([DPC, R], bf16)
            vaug = big.tile([128, (R // KT) * VW], bf16)
            headT = big.tile([DPC, R], bf16)

            # ---- projections ----
            # qT = (Wq/8).T @ xtb + bq/8 ; kT = Wk.T @ ytb + bk (per-partition bias)
            for src, wsb, bias, dst, do_v in ((xtb, wq_sb, bq_sb, qT, False),
                                              (ytb, wk_sb, bk_sb, kT, True)):
                for qc in range(R // 1024):  # 1024-wide chunks for DMA efficiency
                    tiles = []
                    for dt in range(8):
                        t = xy.tile([128, 1024], bf16, tag="xy")
                        nc.sync.dma_start(t[:], src[dt * 128:(dt + 1) * 128,
                                                    qc * 1024:(qc + 1) * 1024])
                        tiles.append(t)
                    for s in range(2):  # 512-wide matmul slices
                        ps = pp.tile([128, QW], f32, tag="pp")
                        for dt in range(8):
                            nc.tensor.matmul(ps[:DPC, :], wsb[:, dt * DPC:(dt + 1) * DPC],
                                             tiles[dt][:, s * QW:(s + 1) * QW],
                                             start=(dt == 0), stop=(dt == 7))
                        col = qc * 1024 + s * QW
                        nc.scalar.activation(dst[:, col:col + QW], ps[:DPC, :],
                                             mybir.ActivationFunctionType.Identity,
                                             bias=bias)
                    if do_v:
                        # v_aug[kt] = [v_h0 | 1 | v_h1 | 1] for the 8 k-tiles in chunk
                        for j in range(8):
                            kti = qc * 8 + j
                            psv = pp.tile([128, QW], f32, tag="pp")
                            for dt in range(8):
                                nc.tensor.matmul(psv[:, :VW],
                                                 tiles[dt][:, j * KT:(j + 1) * KT],
                                                 wv_sb[:, dt * VW:(dt + 1) * VW],
                                                 start=(dt == 0), stop=False)
                            nc.tensor.matmul(psv[:, :VW], ones_row[:],
                                             bv1_sb[:], start=False, stop=True)
                            nc.vector.tensor_copy(vaug[:, kti * VW:kti * VW + VW],
                                                  psv[:, :VW])

            # ---- attention per (batch, local head, q-window) ----
            for n in range(NB):
                for hp in range(HPC):
                    hs = hp * DK  # partition offset of this head in qT/kT
                    for qb in range(NQB):
                        qcol = n * L + qb * QW
                        exp_tiles = {}
                        for kt in range(NQT):
                            cls = blocks[qb][kt]
                            if cls == 0:
                                continue
                            ps = sp.tile([128, QW], f32, tag="sp")
                            nc.tensor.matmul(
                                ps[:KT, :],
                                kT[hs:hs + DK, n * L + kt * KT:n * L + (kt + 1) * KT],
                                qT[hs:hs + DK, qcol:qcol + QW],
                                start=True, stop=True)
                            et = expp.tile([KT, QW], bf16, tag="exp")
                            nc.scalar.activation(et[:], ps[:KT, :],
                                                 mybir.ActivationFunctionType.Exp)
                            if cls == 2:
                                p = pat_idx[qb][kt]
                                nc.vector.tensor_tensor(
                                    et[:], et[:], mpat_sb[:, p * QW:(p + 1) * QW],
                                    mybir.AluOpType.mult)
                            exp_tiles[kt] = et
                        for j in range(QW // KT):
                            qi = qb * (QW // KT) + j
                            kts = [kt for kt in exp_tiles if m128[qi][kt]]
                            if not kts:
                                continue
                            av = avp.tile([128, 65], f32, tag="avp")
                            for i, kt in enumerate(kts):
                                nc.tensor.matmul(
                                    av[:, :], exp_tiles[kt][:, j * KT:(j + 1) * KT],
                                    vaug[:, (n * NQT + kt) * VW + hp * 65:
                                         (n * NQT + kt) * VW + (hp + 1) * 65],
                                    start=(i == 0), stop=(i == len(kts) - 1))
                            rc = sm.tile([128, 1], f32, tag="rc")
                            nc.vector.reciprocal(rc[:], av[:, 64:65])
                            hd = sm.tile([128, DK], bf16, tag="hd")
                            nc.vector.tensor_scalar_mul(hd[:], av[:, 0:DK], rc[:])
                            tp = trp.tile([128, 128], bf16, tag="trp")
                            nc.tensor.transpose(tp[hs:hs + DK, :], hd[:], idn_sb[:])
                            nc.vector.tensor_copy(
                                headT[hs:hs + DK, n * L + qi * KT:n * L + (qi + 1) * KT],
                                tp[hs:hs + DK, :])

            # ---- AllToAll: head-split -> sequence-split ----
            a2a_in = dram.tile([NC, DPC, CHUNK], bf16)
            a2a_out = dram.tile([NC, DPC, CHUNK], bf16)
            for r in range(NC):
                nc.sync.dma_start(a2a_in[r], headT[:, r * CHUNK:(r + 1) * CHUNK])
            nc.gpsimd.collective_compute(
                "AllToAll", mybir.AluOpType.bypass,
                replica_groups=[list(range(NC))],
                ins=[a2a_in.opt()], outs=[a2a_out.opt()])

            # ---- output projection: out_t = Wo.T-contract + bo, transposed ----
            rhs = []
            for jj in range(8):
                t = expp.tile([DPC, CHUNK], bf16, tag="exp")
                nc.sync.dma_start(t[:], a2a_out[jj])
                rhs.append(t)
            for mt in range(8):
                ps = pp.tile([128, QW], f32, tag="pp")
                for jj in range(8):
                    nc.tensor.matmul(ps[:], wo_sb[:, jj * DM + mt * 128:jj * DM + (mt + 1) * 128],
                                     rhs[jj][:], start=(jj == 0), stop=(jj == 7))
                ob = osb.tile([128, CHUNK], f32, tag="osb")
                nc.scalar.activation(ob[:], ps[:],
                                     mybir.ActivationFunctionType.Identity,
                                     bias=bo_sb[:, mt:mt + 1])
                nc.sync.dma_start(out_t[mt * 128:(mt + 1) * 128, :], ob[:])

    nc.compile()
    return nc


def kernel(x, y, mask, Wq, bq, Wk, bk, Wv, bv, Wo, bo, _trace=False):
    x = np.asarray(x, np.float32)
    y = np.asarray(y, np.float32)
    blocks, m128, pats, pat_idx = _classify_blocks(mask)

    key = (x.shape, tuple(tuple(b) for b in blocks), m128.tobytes(), pats.tobytes())
    if key not in _CACHE:
        _CACHE[key] = _build(blocks, m128, pat_idx, pats.shape[0])
    nc = _CACHE[key]

    fac = np.float32(1.0 / np.sqrt(DK))
    xtb = np.ascontiguousarray(
        np.concatenate([x[n].T for n in range(NB)], axis=1)).astype(BF16)
    ytb = np.ascontiguousarray(
        np.concatenate([y[n].T for n in range(NB)], axis=1)).astype(BF16)
    Wq32 = np.asarray(Wq, np.float32) * fac
    bq32 = np.asarray(bq, np.float32) * fac
    idn = np.eye(128, dtype=BF16)

    in_maps = []
    for c in range(NC):
        d0 = c * DPC
        wv_aug = np.zeros((DM, HPC * 65), np.float32)
        bv1 = np.zeros((1, HPC * 65), np.float32)
        for hp in range(HPC):
            h = HPC * c + hp
            wv_aug[:, hp * 65:hp * 65 + DK] = np.asarray(Wv, np.float32)[:, h * DK:(h + 1) * DK]
            bv1[0, hp * 65:hp * 65 + DK] = np.asarray(bv, np.float32)[h * DK:(h + 1) * DK]
            bv1[0, hp * 65 + DK] = 1.0
        in_maps.append({
            "xtb": xtb, "ytb": ytb,
            "wq": Wq32[:, d0:d0 + DPC].astype(BF16),
            "wk": np.asarray(Wk, np.float32)[:, d0:d0 + DPC].astype(BF16),
            "wv": wv_aug.astype(BF16),
            "wo": np.asarray(Wo, np.float32).astype(BF16),
            "bq": bq32[d0:d0 + DPC].reshape(DPC, 1),
            "bk": np.asarray(bk, np.float32)[d0:d0 + DPC].reshape(DPC, 1),
            "bv1": bv1.astype(BF16),
            "bo": np.asarray(bo, np.float32).reshape(DM, 1),
            "mpat": pats,
            "idn": idn,
        })

    res = run_bass_kernel_spmd(nc, in_maps, core_ids=list(range(NC)), trace=_trace)
    out = np.empty((NB, L, DM), np.float32)
    for c in range(NC):
        n = c // (NC // NB)
        q0 = CHUNK * (c % (NC // NB))
        out[n, q0:q0 + CHUNK, :] = res.results[c]["out_t"].T
    if _trace:
        kernel.last_results = res
    return out
